# revision 27
# baseline (speedup 1.0000x reference)
"""ALiBi positional-embedding bias kernel for 8 TRN2 NeuronCores.

Reference math (B=8, H=8, L=1024, TOKEN_NUM=100):
    out[b,h,i,j] = ( tri[i,j] + slopes[h] * base[b,i,j] ) / 5
with tri = -inf on the strict upper triangle (0 elsewhere) and
    base[i,j] = kv[j] + eq[i,j]*(thc[i,j] + resp[j]) - oxth[j]*delta(i,j)
    kv[j]     = j + s2[j] + oxth[j]
    s2[j]     = (101-d[j]) if 101-d[j] > 50.5 else 0
    ox[j]     = 101-d[j] if resp[j]==1 else d[j];  oxth = ox if ox > 50.5 else 0
    eq[i,j]   = [d[i]==d[j]]
    cnt[i,j]  = #{j' <= j : d[j']==d[i]};  thc = cnt if cnt > 102.4 else 0
Since slopes > 0, folding -inf into base before the per-h scale is exact.

Sharding: data-parallel over batch, one batch row per core; slopes
replicated; each core emits its own [H, L, L] block independently.

Layout notes:
 - device output is [L/128, 128, H, L] (row-tile, partition, head, key):
   each row-tile flushes as ONE fully-contiguous 4 MiB DMA; host
   transposes back to [H, L, L].
 - the L-point "row math" (s2/ox/kv chains) runs on a [128, 16]
   transposed layout (cols 0-7: j=8p+f; cols 8-15: j=128(f-8)+p) so all
   128 lanes work instead of broadcast [128, L] tiles; kv feeds back to
   [1, L] bf16 matmul rows via a tiny DRAM scratch round-trip.
"""

import numpy as np

import concourse.bacc as bacc
import concourse.mybir as mybir
import concourse.tile as tile
from concourse.bass_utils import run_bass_kernel_spmd
from concourse.tile_rust import add_dep_helper

B, H, L = 8, 8, 1024
R = L // 128  # row-tiles
TN1 = 101.0  # TOKEN_NUM + 1
N_CORES = 8
FP32 = mybir.dt.float32
BF16 = mybir.dt.bfloat16
I32 = mybir.dt.int32
NEG_INF = float("-inf")

# h -> engine for the per-head scale ops ("a"=ACT, "v"=DVE)
H_ENGINE = ["a", "a", "a", "a", "a", "a", "v", "v"]

_CACHED_NC = None


def build_nc():
    nc = bacc.Bacc("TRN2", target_bir_lowering=False, debug=False,
                   num_devices=N_CORES)

    d_ext = nc.dram_tensor("diff", [1, L], FP32, kind="ExternalInput")
    r_ext = nc.dram_tensor("resp", [1, L], FP32, kind="ExternalInput")
    s_ext = nc.dram_tensor("slopes", [1, H], FP32, kind="ExternalInput")
    out_ext = nc.dram_tensor("out", [R, 128, H, L], FP32, kind="ExternalOutput")

    AL = mybir.AluOpType
    F = L // 128  # 8 columns per transposed layout

    with tile.TileContext(nc) as tc:
        with (
            tc.tile_pool(name="const", bufs=1) as cpool,
            tc.tile_pool(name="work", bufs=3) as wpool,
            tc.tile_pool(name="outp", bufs=3) as opool,
            tc.tile_pool(name="psum", bufs=2, space="PSUM") as ppool,
            tc.tile_pool(name="dram", bufs=1, space="DRAM") as dpool,
        ):
            # ---- tiny transposed input loads (ACT HWDGE ring) ----
            # "pf" layout: j = F*p + f (per-partition contiguous, 32B descs)
            dT = cpool.tile([128, F], FP32)
            rT = cpool.tile([128, F], FP32)
            nc.sync.dma_start(out=dT[:],
                              in_=d_ext[0].rearrange("(p f) -> p f", p=128))
            nc.sync.dma_start(out=rT[:],
                              in_=r_ext[0].rearrange("(p f) -> p f", p=128))

            # full-row broadcasts for onehot/whot (SP ring, in parallel)
            db = cpool.tile([128, L], FP32)
            rb = cpool.tile([128, L], FP32)
            slv = cpool.tile([128, H], FP32)
            nc.scalar.dma_start(out=db[:], in_=d_ext[:].to_broadcast([128, L]))
            nc.scalar.dma_start(out=rb[:], in_=r_ext[:].to_broadcast([128, L]))
            nc.scalar.dma_start(out=slv[:], in_=s_ext[:].to_broadcast([128, H]))

            slv5 = cpool.tile([128, H], FP32)  # slopes / 5, per-partition scalars
            # on ACT so the activation table load happens early (hidden)
            nc.scalar.mul(slv5[:], slv[:], 0.2)

            # ---- row math on the [128, F] transposed tiles (~100ns/op) ----
            de2T = cpool.tile([128, F], FP32)
            nc.vector.tensor_scalar(de2T[:], dT[:], -1.0, TN1,
                                    op0=AL.mult, op1=AL.add)
            s2T = cpool.tile([128, F], FP32)
            nc.vector.scalar_tensor_tensor(s2T[:], de2T[:], 50.5, de2T[:],
                                           op0=AL.is_gt, op1=AL.mult)
            rdT = cpool.tile([128, F], FP32)
            nc.vector.tensor_mul(rdT[:], rT[:], dT[:])
            u1T = cpool.tile([128, F], FP32)
            nc.vector.scalar_tensor_tensor(u1T[:], rdT[:], -2.0, dT[:],
                                           op0=AL.mult, op1=AL.add)
            oxT = cpool.tile([128, F], FP32)
            nc.vector.scalar_tensor_tensor(oxT[:], rT[:], TN1, u1T[:],
                                           op0=AL.mult, op1=AL.add)
            oxthT = cpool.tile([128, F], FP32)
            nc.vector.scalar_tensor_tensor(oxthT[:], oxT[:], 50.5, oxT[:],
                                           op0=AL.is_gt, op1=AL.mult)

            # kv = j + s2 + oxth
            jTi = cpool.tile([128, F], I32)
            nc.gpsimd.iota(jTi[:], pattern=[[1, F]], channel_multiplier=F)
            jT = cpool.tile([128, F], FP32)
            nc.vector.tensor_copy(out=jT[:], in_=jTi[:])
            kpT = cpool.tile([128, F], FP32)
            nc.vector.tensor_add(kpT[:], jT[:], s2T[:])
            kvT = cpool.tile([128, F], FP32)
            nc.vector.tensor_add(kvT[:], kpT[:], oxthT[:])
            # exact bf16 hi/lo split (kv is integer <= ~1224)
            kv_hiT = cpool.tile([128, F], BF16)
            nc.vector.tensor_copy(out=kv_hiT[:], in_=kvT[:])
            kv_hiTf = cpool.tile([128, F], FP32)
            nc.vector.tensor_copy(out=kv_hiTf[:], in_=kv_hiT[:])
            kv_loT = cpool.tile([128, F], BF16)
            kv_tail = nc.vector.tensor_sub(kv_loT[:], kvT[:], kv_hiTf[:])

            # round-trip kv_hi/kv_lo rows and an oxth broadcast via DRAM
            # (sync HWDGE ring; gpsimd SWDGE issue is too slow for this chain)
            scr = dpool.tile([2, L], BF16)
            scro = dpool.tile([1, L], FP32)
            nc.sync.dma_start(
                out=scr[0].rearrange("(p f) -> p f", p=128), in_=kv_hiT[:])
            nc.sync.dma_start(
                out=scr[1].rearrange("(p f) -> p f", p=128), in_=kv_loT[:])
            nc.sync.dma_start(
                out=scro[0].rearrange("(p f) -> p f", p=128), in_=oxthT[:])
            kv_hi = cpool.tile([1, L], BF16)
            kv_lo = cpool.tile([1, L], BF16)
            oxthb = cpool.tile([128, L], FP32)
            nc.sync.dma_start(out=kv_hi[:], in_=scr[0:1, :])
            nc.sync.dma_start(out=kv_lo[:], in_=scr[1:2, :])
            nc.sync.dma_start(out=oxthb[:], in_=scro[:].to_broadcast([128, L]))

            # onehot[v,j] = [d[j] == v]; whot = onehot * resp[j]
            iota_p_i = cpool.tile([128, 1], I32)
            nc.gpsimd.iota(iota_p_i[:], pattern=[[0, 1]], channel_multiplier=1)
            iota_p = cpool.tile([128, 1], FP32)
            nc.vector.tensor_copy(out=iota_p[:], in_=iota_p_i[:])
            onehot = cpool.tile([128, L], BF16)
            i_oh = nc.vector.tensor_scalar(onehot[:], db[:], iota_p[:], None,
                                           op0=AL.is_equal)
            whot = cpool.tile([128, L], BF16)
            i_wh = nc.vector.scalar_tensor_tensor(whot[:], db[:], iota_p[:],
                                                  rb[:],
                                                  op0=AL.is_equal, op1=AL.mult)
            # cumhot[v,j] = cumsum_j onehot[v,j]
            cumhot = cpool.tile([128, L], BF16)
            i_sc = nc.vector.tensor_tensor_scan(cumhot[:], onehot[:],
                                                onehot[:], 0.0,
                                                op0=AL.add, op1=AL.bypass)
            # keep the kv hi/lo tail (which gates the DRAM round-trip and
            # the p_c matmuls) ahead of these long DVE ops in the schedule
            for big in (i_oh, i_wh, i_sc):
                add_dep_helper(big.ins, kv_tail.ins, sync=False,
                               reason="prioritize kv round-trip over bulk DVE")

            ones_row = cpool.tile([1, 128], BF16)
            nc.vector.memset(ones_row[:], 1.0)

            # ---- main loop over 8 row-tiles ----
            for r in range(R):
                r0 = r * 128
                oh_r = onehot[:, r0:r0 + 128]  # stationary [128v, 128i]
                base_t = wpool.tile([128, L], FP32, tag="base")
                # chunks whose columns are all > r0+127 are pure -inf: skip.
                n_chunks = 1 if r0 + 127 < 512 else 2
                W = n_chunks * 512
                for c in range(n_chunks):
                    c0 = c * 512
                    sl_c = slice(c0, c0 + 512)
                    p_eq = ppool.tile([128, 512], FP32, tag="eq")
                    p_cnt = ppool.tile([128, 512], FP32, tag="cnt")
                    p_c = ppool.tile([128, 512], FP32, tag="c")
                    nc.tensor.matmul(p_eq[:], oh_r, onehot[:, sl_c])
                    nc.tensor.matmul(p_cnt[:], oh_r, cumhot[:, sl_c])
                    nc.tensor.matmul(p_c[:], oh_r, whot[:, sl_c],
                                     start=True, stop=False)
                    nc.tensor.matmul(p_c[:], ones_row[:], kv_hi[:, sl_c],
                                     start=False, stop=False)
                    nc.tensor.matmul(p_c[:], ones_row[:], kv_lo[:, sl_c],
                                     start=False, stop=True)
                    # thc = cnt*[cnt>102.4]; s4 = thc*eq; base = s4 + (s5+kv)
                    g2 = wpool.tile([128, 512], FP32, tag="g2")
                    nc.vector.tensor_scalar(g2[:], p_cnt[:], L * 0.1, None,
                                            op0=AL.is_gt)
                    thc = wpool.tile([128, 512], FP32, tag="thc")
                    nc.vector.tensor_mul(thc[:], g2[:], p_cnt[:])
                    s4 = wpool.tile([128, 512], FP32, tag="s4")
                    nc.vector.tensor_mul(s4[:], thc[:], p_eq[:])
                    nc.vector.tensor_add(base_t[:, sl_c], s4[:], p_c[:])

                # diagonal fix: base[p, r0+p] -= oxth[r0+p]
                dsel = wpool.tile([128, 128], FP32, tag="dsel")
                nc.gpsimd.affine_select(
                    dsel[:], oxthb[:, r0:r0 + 128],
                    pattern=[[-1, 128]], compare_op=AL.is_equal, fill=0.0,
                    base=0, channel_multiplier=1,
                )
                nc.vector.tensor_sub(base_t[:, r0:r0 + 128],
                                     base_t[:, r0:r0 + 128], dsel[:])

                # causal mask: -inf where j > r0 + p  (keep where r0+p-j >= 0)
                if W < L:
                    nc.gpsimd.memset(base_t[:, W:], NEG_INF)
                nc.gpsimd.affine_select(
                    base_t[:, :W], base_t[:, :W],
                    pattern=[[-1, W]], compare_op=AL.is_ge, fill=NEG_INF,
                    base=r0, channel_multiplier=1,
                )

                # 8 head planes in four [128, 2, L] quarters, one DMA each
                for q in range(4):
                    o_t = opool.tile([128, 2, L], FP32, tag=f"o{q}")
                    for hh in range(2):
                        h = 2 * q + hh
                        if H_ENGINE[h] == "a":
                            nc.scalar.activation(
                                o_t[:, hh, :], base_t[:],
                                mybir.ActivationFunctionType.Copy,
                                bias=0.0, scale=slv5[:, h:h + 1],
                            )
                        else:
                            nc.vector.tensor_scalar_mul(
                                o_t[:, hh, :], base_t[:], slv5[:, h:h + 1])
                    dma_eng = nc.sync if q % 2 == 0 else nc.scalar
                    dma_eng.dma_start(
                        out=out_ext[r][:, 2 * q:2 * q + 2, :], in_=o_t[:])

    nc.compile()
    return nc


def kernel(tensor=None, slopes=None, diff=None, response=None):
    global _CACHED_NC
    if _CACHED_NC is None:
        _CACHED_NC = build_nc()
    nc = _CACHED_NC

    slopes = np.asarray(slopes, dtype=np.float32).reshape(1, H)
    diff_f = np.asarray(diff, dtype=np.float32)
    resp_f = np.asarray(response, dtype=np.float32)

    in_maps = [
        {
            "diff": np.ascontiguousarray(diff_f[b:b + 1, :]),
            "resp": np.ascontiguousarray(resp_f[b:b + 1, :]),
            "slopes": slopes,
        }
        for b in range(B)
    ]
    res = run_bass_kernel_spmd(nc, in_maps, core_ids=list(range(N_CORES)))
    out = np.empty((B, H, L, L), dtype=np.float32)
    for b in range(B):
        dev = np.asarray(res.results[b]["out"]).reshape(R, 128, H, L)
        out[b] = dev.transpose(2, 0, 1, 3).reshape(H, L, L)
    return out


# revision 28
# speedup vs baseline: 1.2050x; 1.2050x over previous
"""ALiBi positional-embedding bias kernel for 8 TRN2 NeuronCores.

Reference math (B=8, H=8, L=1024, TOKEN_NUM=100):
    out[b,h,i,j] = ( tri[i,j] + slopes[h] * base[b,i,j] ) / 5
with tri = -inf on the strict upper triangle (0 elsewhere) and
    base[i,j] = kv[j] + eq[i,j]*(thc[i,j] + resp[j]) - oxth[j]*delta(i,j)
    kv[j]     = j + s2[j] + oxth[j]
    s2[j]     = (101-d[j]) if 101-d[j] > 50.5 else 0
    ox[j]     = 101-d[j] if resp[j]==1 else d[j];  oxth = ox if ox > 50.5 else 0
    eq[i,j]   = [d[i]==d[j]]
    cnt[i,j]  = #{j' <= j : d[j']==d[i]};  thc = cnt if cnt > 102.4 else 0
Since slopes > 0, folding -inf into base before the per-h scale is exact.

Sharding: data-parallel over batch, one batch row per core; slopes
replicated; each core emits its own [H, L, L] block independently.

Layout notes:
 - device output is [L/128, 128, H, L] (row-tile, partition, head, key);
   each row-tile flushes as four [128, 2, L] quarters, one DMA each,
   alternating between the SP and ACT HWDGE rings; host transposes back
   to [H, L, L].
 - the L-point "row math" (s2/ox/kv chains) runs on a [128, 8]
   transposed layout (j = 8p + f) so all 128 lanes work; kv and oxth
   bounce through DRAM scratch and return as [128, L] broadcasts.
"""

import numpy as np

import concourse.bacc as bacc
import concourse.mybir as mybir
import concourse.tile as tile
from concourse.bass_utils import run_bass_kernel_spmd

B, H, L = 8, 8, 1024
R = L // 128  # row-tiles
TN1 = 101.0  # TOKEN_NUM + 1
N_CORES = 8
FP32 = mybir.dt.float32
BF16 = mybir.dt.bfloat16
I32 = mybir.dt.int32
NEG_INF = float("-inf")

# h -> engine for the per-head scale ops ("a"=ACT, "v"=DVE)
H_ENGINE = ["a", "a", "a", "a", "a", "a", "v", "v"]

_CACHED_NC = None


def build_nc():
    nc = bacc.Bacc("TRN2", target_bir_lowering=False, debug=False,
                   num_devices=N_CORES)

    d_ext = nc.dram_tensor("diff", [1, L], FP32, kind="ExternalInput")
    r_ext = nc.dram_tensor("resp", [1, L], FP32, kind="ExternalInput")
    s_ext = nc.dram_tensor("slopes", [1, H], FP32, kind="ExternalInput")
    out_ext = nc.dram_tensor("out", [R, 128, H, L], FP32, kind="ExternalOutput")

    AL = mybir.AluOpType
    F = L // 128  # 8 columns in the transposed layout

    with tile.TileContext(nc) as tc:
        with (
            tc.tile_pool(name="const", bufs=1) as cpool,
            tc.tile_pool(name="work", bufs=3) as wpool,
            tc.tile_pool(name="outp", bufs=3) as opool,
            tc.tile_pool(name="psum", bufs=2, space="PSUM") as ppool,
            tc.tile_pool(name="dram", bufs=1, space="DRAM") as dpool,
        ):
            # ---- inputs: tiny transposed loads + row broadcasts ----
            dT = cpool.tile([128, F], FP32)
            rT = cpool.tile([128, F], FP32)
            db = cpool.tile([128, L], FP32)
            rb = cpool.tile([128, L], FP32)
            slv = cpool.tile([128, H], FP32)
            nc.sync.dma_start(out=dT[:],
                              in_=d_ext[0].rearrange("(p f) -> p f", p=128))
            nc.sync.dma_start(out=rT[:],
                              in_=r_ext[0].rearrange("(p f) -> p f", p=128))
            nc.scalar.dma_start(out=db[:], in_=d_ext[:].to_broadcast([128, L]))
            nc.sync.dma_start(out=rb[:], in_=r_ext[:].to_broadcast([128, L]))
            nc.scalar.dma_start(out=slv[:], in_=s_ext[:].to_broadcast([128, H]))

            slv5 = cpool.tile([128, H], FP32)  # slopes / 5, per-partition scalars
            # on ACT so the activation table load happens early (hidden)
            nc.scalar.mul(slv5[:], slv[:], 0.2)

            # ---- row math on the [128, F] transposed tiles (~100ns/op) ----
            de2T = cpool.tile([128, F], FP32)
            nc.vector.tensor_scalar(de2T[:], dT[:], -1.0, TN1,
                                    op0=AL.mult, op1=AL.add)
            s2T = cpool.tile([128, F], FP32)
            nc.vector.scalar_tensor_tensor(s2T[:], de2T[:], 50.5, de2T[:],
                                           op0=AL.is_gt, op1=AL.mult)
            rdT = cpool.tile([128, F], FP32)
            nc.vector.tensor_mul(rdT[:], rT[:], dT[:])
            u1T = cpool.tile([128, F], FP32)
            nc.vector.scalar_tensor_tensor(u1T[:], rdT[:], -2.0, dT[:],
                                           op0=AL.mult, op1=AL.add)
            oxT = cpool.tile([128, F], FP32)
            nc.vector.scalar_tensor_tensor(oxT[:], rT[:], TN1, u1T[:],
                                           op0=AL.mult, op1=AL.add)
            oxthT = cpool.tile([128, F], FP32)
            nc.vector.scalar_tensor_tensor(oxthT[:], oxT[:], 50.5, oxT[:],
                                           op0=AL.is_gt, op1=AL.mult)

            # kv = j + s2 + oxth
            jTi = cpool.tile([128, F], I32)
            nc.gpsimd.iota(jTi[:], pattern=[[1, F]], channel_multiplier=F)
            jT = cpool.tile([128, F], FP32)
            nc.vector.tensor_copy(out=jT[:], in_=jTi[:])
            kpT = cpool.tile([128, F], FP32)
            nc.vector.tensor_add(kpT[:], jT[:], s2T[:])
            kvT = cpool.tile([128, F], FP32)
            nc.vector.tensor_add(kvT[:], kpT[:], oxthT[:])

            # bounce kv/oxth rows through DRAM, return as [128, L] broadcasts
            scr = dpool.tile([2, L], FP32)
            nc.sync.dma_start(
                out=scr[0].rearrange("(p f) -> p f", p=128), in_=kvT[:])
            nc.sync.dma_start(
                out=scr[1].rearrange("(p f) -> p f", p=128), in_=oxthT[:])
            kvb = cpool.tile([128, L], FP32)
            oxthb = cpool.tile([128, L], FP32)
            nc.sync.dma_start(out=kvb[:], in_=scr[0:1, :].to_broadcast([128, L]))
            nc.sync.dma_start(out=oxthb[:],
                              in_=scr[1:2, :].to_broadcast([128, L]))

            # onehot[v,j] = [d[j] == v]; whot = onehot * resp[j]
            iota_p_i = cpool.tile([128, 1], I32)
            nc.gpsimd.iota(iota_p_i[:], pattern=[[0, 1]], channel_multiplier=1)
            iota_p = cpool.tile([128, 1], FP32)
            nc.vector.tensor_copy(out=iota_p[:], in_=iota_p_i[:])
            onehot = cpool.tile([128, L], BF16)
            nc.vector.tensor_scalar(onehot[:], db[:], iota_p[:], None,
                                    op0=AL.is_equal)
            whot = cpool.tile([128, L], BF16)
            nc.vector.scalar_tensor_tensor(whot[:], db[:], iota_p[:], rb[:],
                                           op0=AL.is_equal, op1=AL.mult)
            # cumhot[v,j] = cumsum_j onehot[v,j]
            cumhot = cpool.tile([128, L], BF16)
            nc.vector.tensor_tensor_scan(cumhot[:], onehot[:], onehot[:], 0.0,
                                         op0=AL.add, op1=AL.bypass)

            # ---- main loop over 8 row-tiles ----
            for r in range(R):
                r0 = r * 128
                oh_r = onehot[:, r0:r0 + 128]  # stationary [128v, 128i]
                base_t = wpool.tile([128, L], FP32, tag="base")
                # chunks whose columns are all > r0+127 are pure -inf: skip.
                n_chunks = 1 if r0 + 127 < 512 else 2
                W = n_chunks * 512
                for c in range(n_chunks):
                    c0 = c * 512
                    sl_c = slice(c0, c0 + 512)
                    p_eq = ppool.tile([128, 512], FP32, tag="eq")
                    p_cnt = ppool.tile([128, 512], FP32, tag="cnt")
                    p_c = ppool.tile([128, 512], FP32, tag="c")
                    nc.tensor.matmul(p_eq[:], oh_r, onehot[:, sl_c])
                    nc.tensor.matmul(p_cnt[:], oh_r, cumhot[:, sl_c])
                    nc.tensor.matmul(p_c[:], oh_r, whot[:, sl_c])
                    # thc = cnt*[cnt>102.4]; s4 = thc*eq;
                    # base = s4 + (eq*resp) + kv
                    g2 = wpool.tile([128, 512], FP32, tag="g2")
                    nc.vector.tensor_scalar(g2[:], p_cnt[:], L * 0.1, None,
                                            op0=AL.is_gt)
                    thc = wpool.tile([128, 512], FP32, tag="thc")
                    nc.vector.tensor_mul(thc[:], g2[:], p_cnt[:])
                    s4 = wpool.tile([128, 512], FP32, tag="s4")
                    nc.vector.tensor_mul(s4[:], thc[:], p_eq[:])
                    t5 = wpool.tile([128, 512], FP32, tag="t5")
                    nc.vector.tensor_add(t5[:], s4[:], p_c[:])
                    nc.vector.tensor_add(base_t[:, sl_c], t5[:], kvb[:, sl_c])

                # diagonal fix: base[p, r0+p] -= oxth[r0+p]
                dsel = wpool.tile([128, 128], FP32, tag="dsel")
                nc.gpsimd.affine_select(
                    dsel[:], oxthb[:, r0:r0 + 128],
                    pattern=[[-1, 128]], compare_op=AL.is_equal, fill=0.0,
                    base=0, channel_multiplier=1,
                )
                nc.vector.tensor_sub(base_t[:, r0:r0 + 128],
                                     base_t[:, r0:r0 + 128], dsel[:])

                # causal mask: -inf where j > r0 + p  (keep where r0+p-j >= 0)
                if W < L:
                    nc.gpsimd.memset(base_t[:, W:], NEG_INF)
                nc.gpsimd.affine_select(
                    base_t[:, :W], base_t[:, :W],
                    pattern=[[-1, W]], compare_op=AL.is_ge, fill=NEG_INF,
                    base=r0, channel_multiplier=1,
                )

                # 8 head planes in four [128, 2, L] quarters, one DMA each
                for q in range(4):
                    o_t = opool.tile([128, 2, L], FP32, tag=f"o{q}")
                    for hh in range(2):
                        h = 2 * q + hh
                        if H_ENGINE[h] == "a":
                            nc.scalar.activation(
                                o_t[:, hh, :], base_t[:],
                                mybir.ActivationFunctionType.Copy,
                                bias=0.0, scale=slv5[:, h:h + 1],
                            )
                        else:
                            nc.vector.tensor_scalar_mul(
                                o_t[:, hh, :], base_t[:], slv5[:, h:h + 1])
                    dma_eng = nc.sync if q % 2 == 0 else nc.scalar
                    dma_eng.dma_start(
                        out=out_ext[r][:, 2 * q:2 * q + 2, :], in_=o_t[:])

    nc.compile()
    return nc


def kernel(tensor=None, slopes=None, diff=None, response=None):
    global _CACHED_NC
    if _CACHED_NC is None:
        _CACHED_NC = build_nc()
    nc = _CACHED_NC

    slopes = np.asarray(slopes, dtype=np.float32).reshape(1, H)
    diff_f = np.asarray(diff, dtype=np.float32)
    resp_f = np.asarray(response, dtype=np.float32)

    in_maps = [
        {
            "diff": np.ascontiguousarray(diff_f[b:b + 1, :]),
            "resp": np.ascontiguousarray(resp_f[b:b + 1, :]),
            "slopes": slopes,
        }
        for b in range(B)
    ]
    res = run_bass_kernel_spmd(nc, in_maps, core_ids=list(range(N_CORES)))
    out = np.empty((B, H, L, L), dtype=np.float32)
    for b in range(B):
        dev = np.asarray(res.results[b]["out"]).reshape(R, 128, H, L)
        out[b] = dev.transpose(2, 0, 1, 3).reshape(H, L, L)
    return out
